# revision 9
# baseline (speedup 1.0000x reference)
"""Dark channel prior (15x15 sliding-window min, SAME zero padding) on 8 trn2 cores.

Input  [32, 512, 512, 3] f32, output same shape.
Sharding: pure data parallel, 4 images per core.

Per-core pipeline (all stages resident in SBUF):
  load natural [128 rows, 1536] tiles
  -> horizontal min tree (doubling: shifts 1,2,4,7 pixels) on DVE
  -> PE transpose (identity matmul) -> PSUM -> ScalarE copy to [wc, h] tiles
  -> vertical min tree along free dim
  -> PE transpose back -> natural out tiles -> store.

Border outputs (rows/cols within 7 of an edge) include the zero padding and the
input is non-negative, so they are exactly 0 -> memset, interior computed exactly.
"""

import sys

sys.path.insert(0, "/opt/trn_rl_repo")

import numpy as np

N_CORES = 8
B, H, W, C = 32, 512, 512, 3
WC = W * C  # 1536
K = 15
R = K // 2  # 7
IMGS_PER_CORE = B // N_CORES  # 4
ROWS_PER_CORE = IMGS_PER_CORE * H  # 2048

_BUILD_CACHE = {}


def _emit_image(nc, mybir, img, x, y, ident, pools):
    AluOp = mybir.AluOpType
    f32 = mybir.dt.float32
    xp = pools["xp"]
    hp = pools["hp"]
    mp = pools["mp"]
    vmp = pools["vmp"]
    tp = pools["tp"]
    vp = pools["vp"]
    op_ = pools["op_"]
    ps = pools["ps"]
    ps2 = pools["ps2"]
    r0 = img * H

    # ---- load + horizontal pass (rows on partitions) ----
    hts = []
    for b in range(4):
        xt = xp.tile([128, WC], f32, tag="xt", name=f"xt{img}_{b}")
        nc.sync.dma_start(xt[:], x.ap()[r0 + 128 * b : r0 + 128 * (b + 1), :])
        ht = hp.tile([128, WC], f32, tag="ht", name=f"ht{img}_{b}")
        m2 = mp.tile([128, 1533], f32, tag="m", name=f"m2_{img}_{b}")
        m4 = mp.tile([128, 1527], f32, tag="m", name=f"m4_{img}_{b}")
        m8 = mp.tile([128, 1515], f32, tag="m", name=f"m8_{img}_{b}")
        nc.vector.tensor_tensor(m2[:], xt[:, 0:1533], xt[:, 3:1536], AluOp.min)
        nc.vector.tensor_tensor(m4[:], m2[:, 0:1527], m2[:, 6:1533], AluOp.min)
        nc.vector.tensor_tensor(m8[:], m4[:, 0:1515], m4[:, 12:1527], AluOp.min)
        nc.vector.tensor_tensor(
            ht[:, 21:1515], m8[:, 0:1494], m8[:, 21:1515], AluOp.min
        )
        nc.gpsimd.memset(ht[:, 0:21], 0.0)
        nc.gpsimd.memset(ht[:, 1515:1536], 0.0)
        hts.append(ht)

    # ---- transpose H [512, 1536] -> T [12 blocks][128 wc, 512 h] ----
    timg = tp.tile([128, 12, 512], f32, tag="timg", name=f"timg{img}")
    for c in range(12):
        pt = ps.tile([128, 512], f32, tag="ps", name=f"pt{img}_{c}")
        for r in range(4):
            nc.tensor.transpose(
                pt[:, 128 * r : 128 * (r + 1)],
                hts[r][:, 128 * c : 128 * (c + 1)],
                ident[:],
            )
        nc.scalar.copy(timg[:, c, :], pt[:])

    # ---- vertical pass on transposed tiles (h on free dim) ----
    vimg = vp.tile([128, 12, 512], f32, tag="vimg", name=f"vimg{img}")
    n_groups = pools.get("v_groups", 3)
    gw = 12 // n_groups
    for g in range(n_groups):
        cs = slice(gw * g, gw * (g + 1))
        v2 = vmp.tile([128, gw, 511], f32, tag="vm", name=f"v2_{img}_{g}")
        v4 = vmp.tile([128, gw, 509], f32, tag="vm", name=f"v4_{img}_{g}")
        v8 = vmp.tile([128, gw, 505], f32, tag="vm", name=f"v8_{img}_{g}")
        nc.vector.tensor_tensor(
            v2[:], timg[:, cs, 0:511], timg[:, cs, 1:512], AluOp.min
        )
        nc.vector.tensor_tensor(v4[:], v2[:, :, 0:509], v2[:, :, 2:511], AluOp.min)
        nc.vector.tensor_tensor(v8[:], v4[:, :, 0:505], v4[:, :, 4:509], AluOp.min)
        nc.vector.tensor_tensor(
            vimg[:, cs, 7:505], v8[:, :, 0:498], v8[:, :, 7:505], AluOp.min
        )
        nc.gpsimd.memset(vimg[:, cs, 0:7], 0.0)
        nc.gpsimd.memset(vimg[:, cs, 505:512], 0.0)

    # ---- transpose back + store ----
    for b in range(4):
        ot = op_.tile([128, WC], f32, tag="ot", name=f"ot{img}_{b}")
        for g in range(3):
            pt2 = ps2.tile([128, 512], f32, tag="ps2", name=f"pt2_{img}_{b}_{g}")
            for k in range(4):
                c = 4 * g + k
                nc.tensor.transpose(
                    pt2[:, 128 * k : 128 * (k + 1)],
                    vimg[:, c, 128 * b : 128 * (b + 1)],
                    ident[:],
                )
            nc.scalar.copy(ot[:, 512 * g : 512 * (g + 1)], pt2[:])
        nc.sync.dma_start(y.ap()[r0 + 128 * b : r0 + 128 * (b + 1), :], ot[:])


DEFAULT_BUFS = dict(xp=3, hp=5, mp=2, vmp=2, tp=1, vp=1, op=4, ps=3, ps2=3)


def _build(n_imgs=IMGS_PER_CORE, repeat=1, bufs=None):
    """Build the per-core bass program. Returns the finalized Bacc module.

    repeat>1 wraps the pipeline in an on-device For_i loop (steady-state
    wall-clock timing; output unchanged since each iteration recomputes it).
    """
    bufs = {**DEFAULT_BUFS, **(bufs or {})}
    key = (n_imgs, repeat, tuple(sorted(bufs.items())))
    if key in _BUILD_CACHE:
        return _BUILD_CACHE[key]

    from contextlib import ExitStack, nullcontext

    import concourse.bacc as bacc
    import concourse.tile as tile
    from concourse import mybir
    from concourse.bass_interp import get_hw_module

    f32 = mybir.dt.float32
    rows_total = n_imgs * H

    nc = bacc.Bacc(
        "TRN2", target_bir_lowering=False, debug=False, num_devices=N_CORES
    )
    x = nc.dram_tensor("x", [rows_total, WC], f32, kind="ExternalInput")
    y = nc.dram_tensor("y", [rows_total, WC], f32, kind="ExternalOutput")
    ident_dram = nc.inline_tensor(np.eye(128, dtype=np.float32), name="ident")

    with tile.TileContext(nc) as tc, ExitStack() as ctx:
        cpool = ctx.enter_context(tc.tile_pool(name="const", bufs=1))
        pools = dict(
            xp=ctx.enter_context(tc.tile_pool(name="xp", bufs=bufs["xp"])),
            hp=ctx.enter_context(tc.tile_pool(name="hp", bufs=bufs["hp"])),
            mp=ctx.enter_context(tc.tile_pool(name="mp", bufs=bufs["mp"])),
            vmp=ctx.enter_context(tc.tile_pool(name="vmp", bufs=bufs["vmp"])),
            tp=ctx.enter_context(tc.tile_pool(name="tp", bufs=bufs["tp"])),
            vp=ctx.enter_context(tc.tile_pool(name="vp", bufs=bufs["vp"])),
            op_=ctx.enter_context(tc.tile_pool(name="op", bufs=bufs["op"])),
            ps=ctx.enter_context(
                tc.tile_pool(name="ps", bufs=bufs["ps"], space="PSUM")
            ),
            ps2=ctx.enter_context(
                tc.tile_pool(name="ps2", bufs=bufs["ps2"], space="PSUM")
            ),
        )

        ident = cpool.tile([128, 128], f32)
        nc.sync.dma_start(ident[:], ident_dram.ap())

        loop_cm = tc.For_i(0, repeat, 1) if repeat > 1 else nullcontext()
        with loop_cm:
            for img in range(n_imgs):
                _emit_image(nc, mybir, img, x, y, ident, pools)

    nc.finalize()
    nc.m = get_hw_module(nc.m)
    _BUILD_CACHE[key] = nc
    return nc


def run_sharded(full_input, n_imgs=IMGS_PER_CORE, repeat=1, **kw):
    """full_input: [n_imgs*8, H, W, C]. Returns (full_output, BassKernelResults)."""
    from concourse.bass_utils import run_bass_kernel_spmd

    nc = _build(n_imgs=n_imgs, repeat=repeat)
    xs = np.ascontiguousarray(full_input, dtype=np.float32).reshape(
        N_CORES, n_imgs * H, WC
    )
    in_maps = [{"x": xs[i]} for i in range(N_CORES)]
    res = run_bass_kernel_spmd(nc, in_maps, list(range(N_CORES)), **kw)
    out = np.stack([res.results[i]["y"] for i in range(N_CORES)])
    return out.reshape(N_CORES * n_imgs, H, W, C), res


def kernel(inputs: np.ndarray) -> np.ndarray:
    out, _ = run_sharded(np.asarray(inputs))
    return out.astype(np.float32)
